# revision 12
# baseline (speedup 1.0000x reference)
"""Multi-head attention Trainium2 Bass kernel.

Shapes (hardcoded): B=4, T=2048, E=1024, H=16, DK=64.
Sharding over 8 cores: core c -> (batch b = c//2, head-group g = c%2).
Each core computes 8 heads of one batch end-to-end and a partial output
projection; the host sums the two partials per batch.

Layout strategy (everything transposed so no on-device transposes):
  - inputs fed as x^T [E, T] (host-transposed)
  - Q^T, K^T kept as [65, T] per head (rows 0..63 = head dims; row 64 =
    ones for Q / scaled mask bias for K, so the scores matmul computes
    s + maskbias in one 65-deep contraction)
  - V kept natural [T, f_local], stored per-head 72 cols wide with an
    appended ones-column so attn@V also produces softmax row-sums
  - S^T [keys, q] per (head, k-tile-pair, q-chunk); exp fused with
    1/sqrt(dk) scale on the scalar engine
  - normalize via reciprocal_approx_fast + gpsimd partition_broadcast
    (row-sums staged to a partition-0 tile: custom-DVE ops break on
    partition-offset inputs)
  - output projection consumes x^T_local directly as lhsT; partials
    stored bf16 (host sums in f32)

Engine assignment: PE = matmuls, ACT = exp only, DVE = psum drains +
recip + normalize-mul, Pool = broadcasts + memsets + input-DMA issue.
Schedule: V-proj and KQ-proj(pair0) lead; KQ pairs 1-3 interleave into
the q-half-0 attention sweep; O-proj tiles interleave into half-1,
which runs chunk-outer with per-chunk normalize so only 4 j-tiles of
O-proj remain after the last exp.
"""

import numpy as np

import concourse.bass as bass
import concourse.tile as tile
from concourse import bacc, mybir
from concourse.bass_utils import run_bass_kernel_spmd

F32 = mybir.dt.float32

B, T, E, H = 4, 2048, 1024, 16
DK = E // H            # 64
N_CORES = 8
FL = 512               # local f (8 heads * 64)
HL = 8                 # heads per core
NT = T // 128          # 16 t-tiles
NE = E // 128          # 8 e-tiles
NFT = FL // 128        # 4 local f-tiles (head pairs)
VW = 72                # per-head width in vt tiles (64 data + 1 ones + pad)

BF16 = mybir.dt.bfloat16
DT = BF16
Exp = mybir.ActivationFunctionType.Exp
MUL = mybir.AluOpType.mult


def build_nc():
    nc = bacc.Bacc("TRN2", target_bir_lowering=False, debug=False,
                   enable_asserts=False)

    qT = nc.dram_tensor("qT", [E, T], DT, kind="ExternalInput").ap()
    kT = nc.dram_tensor("kT", [E, T], DT, kind="ExternalInput").ap()
    vT = nc.dram_tensor("vT", [E, T], DT, kind="ExternalInput").ap()
    wqT = nc.dram_tensor("wqT", [E, FL], DT, kind="ExternalInput").ap()
    wkT = nc.dram_tensor("wkT", [E, FL], DT, kind="ExternalInput").ap()
    wvT = nc.dram_tensor("wvT", [E, FL], DT, kind="ExternalInput").ap()
    woT = nc.dram_tensor("woT", [FL, E], DT, kind="ExternalInput").ap()
    bq = nc.dram_tensor("bq", [128, NFT], F32, kind="ExternalInput").ap()
    bk = nc.dram_tensor("bk", [128, NFT], F32, kind="ExternalInput").ap()
    bv = nc.dram_tensor("bv", [1, FL], DT, kind="ExternalInput").ap()
    bo = nc.dram_tensor("bo", [1, E], DT, kind="ExternalInput").ap()
    ones_d = nc.dram_tensor("ones_d", [1, 128], DT, kind="ExternalInput").ap()
    onesrow = nc.dram_tensor("onesrow", [1, T], DT,
                             kind="ExternalInput").ap()
    maskrow = nc.dram_tensor("maskrow", [1, T], DT,
                             kind="ExternalInput").ap()
    out = nc.dram_tensor("out", [T, E], DT, kind="ExternalOutput").ap()

    with tile.TileContext(nc) as tc:
        with (
            tc.tile_pool(name="const", bufs=1) as constp,
            tc.tile_pool(name="qkt", bufs=1) as qktp,
            tc.tile_pool(name="vsb", bufs=1) as vsbp,
            tc.tile_pool(name="xtl", bufs=1) as xtlp,
            tc.tile_pool(name="wqk", bufs=1) as wp,
            tc.tile_pool(name="wo", bufs=1) as wop,
            tc.tile_pool(name="xload", bufs=24) as xlp,
            tc.tile_pool(name="vload", bufs=8) as vlp,
            tc.tile_pool(name="exps", bufs=4) as expp,
            tc.tile_pool(name="norm", bufs=2) as normp,
            tc.tile_pool(name="osb", bufs=4) as osbp,
            tc.tile_pool(name="ps_s", bufs=2, space="PSUM") as ps_s,
            tc.tile_pool(name="ps_o", bufs=2, space="PSUM") as ps_o,
            tc.tile_pool(name="ps_w", bufs=2, space="PSUM") as ps_w,
        ):
            # ---- constants ----
            bq_sb = constp.tile([128, NFT], F32, tag="bq")
            nc.sync.dma_start(out=bq_sb[:], in_=bq)
            bk_sb = constp.tile([128, NFT], F32, tag="bk")
            nc.sync.dma_start(out=bk_sb[:], in_=bk)
            bv_sb = constp.tile([1, FL], DT, tag="bv")
            nc.sync.dma_start(out=bv_sb[:], in_=bv)
            bo_sb = constp.tile([1, E], DT, tag="bo")
            nc.sync.dma_start(out=bo_sb[:], in_=bo)
            ones_sb = constp.tile([1, 128], DT, tag="ones")
            nc.sync.dma_start(out=ones_sb[:], in_=ones_d)

            # persistent per-head Q^T/K^T tiles: rows 0..63 = head data,
            # row 64 = ones (Q) / scaled mask bias (K)
            qt = [qktp.tile([65, T], DT, tag=f"qt{i}", name=f"qt{i}")
                  for i in range(HL)]
            kt = [qktp.tile([65, T], DT, tag=f"kt{i}", name=f"kt{i}")
                  for i in range(HL)]
            for i in range(HL):
                nc.sync.dma_start(out=qt[i][64:65, :], in_=onesrow)
                nc.sync.dma_start(out=kt[i][64:65, :], in_=maskrow)
            # V per t-tile: [128, 8 heads * 72]; per head: cols 0..63 = V,
            # col 64 = 1.0 (row-sum trick), cols 65..71 unused
            vt = [vsbp.tile([128, HL * VW], DT, tag=f"v{j}", name=f"v{j}")
                  for j in range(NT)]
            xtl = [xtlp.tile([128, T], DT, tag=f"x{i}", name=f"x{i}")
                   for i in range(NFT)]

            # ---- weights ----
            w_sb = {}
            for name, wdram in (("k", wkT), ("q", wqT)):
                w_sb[name] = [
                    wp.tile([128, FL], DT, tag=f"w{name}{e}",
                            name=f"w{name}{e}") for e in range(NE)]
                for e in range(NE):
                    nc.sync.dma_start(
                        out=w_sb[name][e][:],
                        in_=wdram[e * 128:(e + 1) * 128, :])
            wv_sb = [wp.tile([128, FL], DT, tag=f"wv{e}", name=f"wv{e}")
                     for e in range(NE)]
            for e in range(NE):
                nc.sync.dma_start(out=wv_sb[e][:],
                                  in_=wvT[e * 128:(e + 1) * 128, :])
            wo_sb = [wop.tile([128, E], DT, tag=f"wo{e}", name=f"wo{e}")
                     for e in range(NFT)]
            for e in range(NFT):
                nc.sync.dma_start(out=wo_sb[e][:],
                                  in_=woT[e * 128:(e + 1) * 128, :])

            # ---------- emission helpers ----------

            def v_proj_units():
                """V projection, one yield-unit per t-tile. vT streamed
                in [128,512] quarters, DMA issued from the Pool queue."""
                for qtr in range(4):
                    vf = [vlp.tile([128, 512], DT, tag="vf", name="vf")
                          for _ in range(NE)]
                    for e in range(NE):
                        nc.gpsimd.dma_start(
                            out=vf[e][:],
                            in_=vT[e * 128:(e + 1) * 128,
                                   qtr * 512:(qtr + 1) * 512])
                    for jj in range(4):
                        j = qtr * 4 + jj
                        ps = ps_w.tile([128, 512], F32, tag="psw",
                                       name="psv")
                        for e in range(NE):
                            nc.tensor.matmul(
                                ps[:],
                                lhsT=vf[e][:, jj * 128:(jj + 1) * 128],
                                rhs=wv_sb[e][:],
                                start=(e == 0), stop=False)
                        nc.tensor.matmul(ps[:], lhsT=ones_sb[:],
                                         rhs=bv_sb[:], start=False, stop=True)
                        nc.vector.tensor_copy(
                            out=vt[j].rearrange(
                                "p (h w) -> p h w", w=VW)[:, :, 0:64],
                            in_=ps[:].rearrange(
                                "p (h w) -> p h w", w=64))
                        nc.gpsimd.memset(
                            vt[j].rearrange(
                                "p (h w) -> p h w", w=VW)[:, :, 64:65],
                            1.0)
                        yield

            def kq_proj_units(fs):
                """K/Q projection, pair-major (pair p completes before
                pair p+1 starts; x chunks re-streamed per pair). DMAs for
                group (f,c+1) are issued before group (f,c)'s units so
                loads stay a group ahead. One yield-unit = one (f, c,
                name) psum tile."""
                groups = [(f, c) for f in fs for c in range(4)]

                def load(f, c):
                    xs = {}
                    for nm, xdram in (("k", kT), ("q", qT)):
                        xs[nm] = []
                        for e in range(NE):
                            xe = xlp.tile([128, 512], DT, tag="xchunk",
                                          name="xchunk")
                            nc.gpsimd.dma_start(
                                out=xe[:],
                                in_=xdram[e * 128:(e + 1) * 128,
                                          c * 512:(c + 1) * 512])
                            xs[nm].append(xe)
                    return xs

                pending = load(*groups[0])
                for gi, (f, c) in enumerate(groups):
                    xs = pending
                    if gi + 1 < len(groups):
                        pending = load(*groups[gi + 1])
                    for nm in ("k", "q"):
                        bias_sb = bk_sb if nm == "k" else bq_sb
                        dsts = kt if nm == "k" else qt
                        ps = ps_w.tile([128, 512], F32, tag="psw",
                                       name="psw")
                        for e in range(NE):
                            nc.tensor.matmul(
                                ps[:],
                                lhsT=w_sb[nm][e][:, f * 128:(f + 1) * 128],
                                rhs=xs[nm][e][:],
                                start=(e == 0), stop=(e == NE - 1))
                        for hh in range(2):
                            nc.vector.tensor_scalar_add(
                                dsts[2 * f + hh][0:64,
                                                 c * 512:(c + 1) * 512],
                                ps[hh * 64:(hh + 1) * 64, :],
                                bias_sb[hh * 64:(hh + 1) * 64, f:f + 1])
                        yield

            def o_proj_units(js):
                """Output projection. One yield-unit = one (j, c2); the
                bf16 partial store DMA is issued per 512-col chunk."""
                for j in js:
                    for c2 in range(2):
                        ps = ps_w.tile([128, 512], F32, tag="psw",
                                       name="psf")
                        nc.tensor.matmul(
                            ps[:], lhsT=ones_sb[:],
                            rhs=bo_sb[:, c2 * 512:(c2 + 1) * 512],
                            start=True, stop=False)
                        for e in range(NFT):
                            nc.tensor.matmul(
                                ps[:],
                                lhsT=xtl[e][:, j * 128:(j + 1) * 128],
                                rhs=wo_sb[e][:, c2 * 512:(c2 + 1) * 512],
                                start=False, stop=(e == NFT - 1))
                        ob = osbp.tile([128, 512], DT, tag="ob", name="ob")
                        nc.vector.tensor_copy(out=ob[:], in_=ps[:])
                        nc.sync.dma_start(
                            out=out[j * 128:(j + 1) * 128,
                                    c2 * 512:(c2 + 1) * 512],
                            in_=ob[:])
                        yield

            # ---- attention: software-pipelined scores/exp/av stream ----
            # one unit = (head, global q-chunk gc, k-tile-pair kp):
            # 2 score matmuls [128,512] into one [128,1024] psum tile,
            # 1 exp, 2 av matmuls accumulating into pso [65,512].

            def att_chunk_units(h, gc):
                qh = qt[h]
                kh = kt[h]
                q0 = gc * 512
                pso = ps_o.tile([65, 512], F32, tag="pso", name="pso")
                for kp in range(NT // 2):
                    st = {}

                    def scores(kp=kp, st=st):
                        ps = ps_s.tile([128, 1024], F32, tag="ps_s",
                                       name="pss")
                        for i in range(2):
                            k = 2 * kp + i
                            nc.tensor.matmul(
                                ps[:, i * 512:(i + 1) * 512],
                                lhsT=kh[:, k * 128:(k + 1) * 128],
                                rhs=qh[:, q0:q0 + 512],
                                start=True, stop=True)
                        st["ps"] = ps

                    def exp_av(kp=kp, st=st, pso=pso, h=h):
                        es = expp.tile([128, 1024], DT, tag="es",
                                       name="es")
                        nc.scalar.activation(
                            out=es[:], in_=st["ps"][:], func=Exp,
                            scale=0.125)
                        for i in range(2):
                            k = 2 * kp + i
                            nc.tensor.matmul(
                                pso[:],
                                lhsT=vt[k][:, h * VW:h * VW + 65],
                                rhs=es[:, i * 512:(i + 1) * 512],
                                start=(kp == 0 and i == 0),
                                stop=(kp == NT // 2 - 1 and i == 1))
                    yield scores, exp_av, pso

            def normalize_cols(h, q0, w, oT, zraw):
                """xtl[...] <- oT[:, :w] / zraw[:, :w], written to q-cols
                q0:q0+w. zraw is partition-0 based (custom-DVE recip
                breaks on partition-offset inputs)."""
                rep = normp.tile([64, 1024], F32, tag="rep", name="rep")
                nc.vector.reciprocal_approx_fast(rep[0:1, 0:w],
                                                 zraw[:, 0:w])
                nc.gpsimd.partition_broadcast(rep[:, 0:w], rep[0:1, 0:w])
                f = h // 2
                r0 = (h % 2) * 64
                nc.vector.tensor_tensor(
                    out=xtl[f][r0:r0 + 64, q0:q0 + w],
                    in0=oT[:, 0:w], in1=rep[:, 0:w], op=MUL)

            # pipeline driver: emits scores(u), then exp+av(u-1), then
            # filler PE units at their rates.
            class Pipe:
                def __init__(self):
                    self.prev = None

                def run_unit(self, scores, tail, fillers, credit):
                    scores()
                    if self.prev is not None:
                        self.prev()
                    for fi, (fill, rate) in enumerate(fillers):
                        credit[fi] += rate
                        while credit[fi] >= 1.0:
                            credit[fi] -= 1.0
                            try:
                                next(fill)
                            except StopIteration:
                                credit[fi] = 0.0
                                break
                    self.prev = tail

                def flush(self):
                    if self.prev is not None:
                        self.prev()
                    self.prev = None

            def sweep_chunk(pipe, h, gc, oT, zraw, cc, fillers, credit):
                """Emit one (head, chunk) group of 8 units; stage pso
                into oT[:, cc*512:...] / zraw[:, cc*512:...] after the
                last unit's exp+av."""
                units = list(att_chunk_units(h, gc))
                for kp, (scores, exp_av, pso) in enumerate(units):
                    if kp == NT // 2 - 1:
                        def tail(ea=exp_av, pso=pso, cc=cc, oT=oT,
                                 zraw=zraw):
                            ea()
                            nc.vector.tensor_copy(
                                out=oT[:, cc * 512:(cc + 1) * 512],
                                in_=pso[0:64, :])
                            nc.vector.tensor_copy(
                                out=zraw[:, cc * 512:(cc + 1) * 512],
                                in_=pso[64:65, :])
                    else:
                        tail = exp_av
                    pipe.run_unit(scores, tail, fillers, credit)

            # ---------- schedule ----------
            # Phase 0: V-proj + KQ pair 0 lead (their DMAs start first;
            # the timing-driven tile scheduler overlaps PE work with the
            # DMA-paced lead-in).
            for _ in v_proj_units():
                pass
            for _ in kq_proj_units([0]):
                pass

            pipe = Pipe()
            # Phase 1: q-half 0 (chunks 0,1), head-outer, batched
            # normalize; KQ pairs 1-3 interleave (pair p done well
            # before head 2p starts).
            f_kq = kq_proj_units([1, 2, 3])
            fillers = [(f_kq, 0.4)]
            credit = [0.0]
            for h in range(HL):
                oT = normp.tile([64, 1024], F32, tag="oT", name="oT")
                zraw = normp.tile([1, 1024], F32, tag="zraw", name="zraw")
                for gc in (0, 1):
                    sweep_chunk(pipe, h, gc, oT, zraw, gc, fillers, credit)
                pipe.flush()
                normalize_cols(h, 0, 1024, oT, zraw)
            for _ in f_kq:
                pass

            # Phase 2: q-half 1, chunk-outer with per-chunk normalize;
            # O-proj j0-7 (half-0 cols) fills chunk-2's sweep, j8-11
            # fills chunk-3's, j12-15 is the only tail.
            f_o1 = o_proj_units(range(8))
            fillers = [(f_o1, 0.25)]
            credit = [0.0]
            for h in range(HL):
                oT = normp.tile([64, 1024], F32, tag="oT", name="oT")
                zraw = normp.tile([1, 1024], F32, tag="zraw", name="zraw")
                sweep_chunk(pipe, h, 2, oT, zraw, 0, fillers, credit)
                pipe.flush()
                normalize_cols(h, 1024, 512, oT, zraw)
            for _ in f_o1:
                pass

            f_o2 = o_proj_units(range(8, 12))
            fillers = [(f_o2, 0.125)]
            credit = [0.0]
            for h in range(HL):
                oT = normp.tile([64, 1024], F32, tag="oT", name="oT")
                zraw = normp.tile([1, 1024], F32, tag="zraw", name="zraw")
                sweep_chunk(pipe, h, 3, oT, zraw, 0, fillers, credit)
                pipe.flush()
                normalize_cols(h, 1536, 512, oT, zraw)
            for _ in f_o2:
                pass
            for _ in o_proj_units(range(12, NT)):
                pass

    nc.compile()
    return nc


_NC_CACHE = None


def _get_nc():
    global _NC_CACHE
    if _NC_CACHE is None:
        _NC_CACHE = build_nc()
    return _NC_CACHE


def make_in_maps(query, key_, value, mask, w_q, b_q, w_k, b_k, w_v, b_v,
                 w_o, b_o):
    import ml_dtypes
    f32 = np.float32
    bf16 = ml_dtypes.bfloat16
    c = lambda a: np.ascontiguousarray(a).astype(bf16)
    in_maps = []
    for core in range(N_CORES):
        b, g = core // 2, core % 2
        fs = slice(g * FL, (g + 1) * FL)
        # row-64 mask bias, pre-multiplied by 8 (exp applies scale 1/8)
        mrow = np.where(mask[b], 0.0, -240.0).astype(f32)
        in_maps.append({
            "qT": c(query[b].T.astype(f32, copy=False)),
            "kT": c(key_[b].T.astype(f32, copy=False)),
            "vT": c(value[b].T.astype(f32, copy=False)),
            "wqT": c(w_q[fs, :].T.astype(f32, copy=False)),
            "wkT": c(w_k[fs, :].T.astype(f32, copy=False)),
            "wvT": c(w_v[fs, :].T.astype(f32, copy=False)),
            "woT": c(w_o[:, fs].T.astype(f32, copy=False)),
            "bq": np.ascontiguousarray(
                b_q[fs].astype(f32, copy=False).reshape(NFT, 128).T),
            "bk": np.ascontiguousarray(
                b_k[fs].astype(f32, copy=False).reshape(NFT, 128).T),
            "bv": b_v[fs].reshape(1, FL).astype(bf16),
            "bo": (b_o.astype(f32, copy=False) if g == 0
                   else np.zeros(E, f32)).reshape(1, E).astype(bf16),
            "ones_d": np.ones((1, 128), bf16),
            "onesrow": np.ones((1, T), bf16),
            "maskrow": mrow.reshape(1, T).astype(bf16),
        })
    return in_maps


def kernel(query=None, key_=None, value=None, mask=None, w_q=None, b_q=None,
           w_k=None, b_k=None, w_v=None, b_v=None, w_o=None, b_o=None,
           key=None, **_kwargs):
    if key_ is None:
        key_ = key
    args = [np.asarray(a) for a in
            (query, key_, value, mask, w_q, b_q, w_k, b_k, w_v, b_v,
             w_o, b_o)]
    nc = _get_nc()
    in_maps = make_in_maps(*args)
    res = run_bass_kernel_spmd(nc, in_maps, core_ids=list(range(N_CORES)))
    outs = [np.asarray(res.results[i]["out"], dtype=np.float32)
            for i in range(N_CORES)]
    full = np.empty((B, T, E), np.float32)
    for b in range(B):
        full[b] = outs[2 * b] + outs[2 * b + 1]
    return full


# revision 15
# speedup vs baseline: 1.0189x; 1.0189x over previous
"""Multi-head attention Trainium2 Bass kernel.

Shapes (hardcoded): B=4, T=2048, E=1024, H=16, DK=64.
Sharding over 8 cores: core c -> (batch b = c//2, head-group g = c%2).
Each core computes 8 heads of one batch end-to-end and a partial output
projection; the host sums the two partials per batch.

Layout strategy (everything transposed so no on-device transposes):
  - inputs fed as x^T [E, T] (host-transposed)
  - Q^T, K^T kept as [65, T] per head (rows 0..63 = head dims; row 64 =
    ones for Q / scaled mask bias for K, so the scores matmul computes
    s + maskbias in one 65-deep contraction)
  - V kept natural [T, f_local], stored per-head 72 cols wide with an
    appended ones-column so attn@V also produces softmax row-sums
  - S^T [keys, q] per (head, k-tile-pair, q-chunk); exp fused with
    1/sqrt(dk) scale on the scalar engine
  - normalize via reciprocal_approx_fast + gpsimd partition_broadcast
    (row-sums staged to a partition-0 tile: custom-DVE ops break on
    partition-offset inputs)
  - output projection consumes x^T_local directly as lhsT; partials
    stored bf16 (host sums in f32)

Engine assignment: PE = matmuls, ACT = exp only, DVE = psum drains +
recip + normalize-mul, Pool = broadcasts + memsets + input-DMA issue.
Schedule: V-proj and KQ-proj(pair0) lead; KQ pairs 1-3 interleave into
the q-half-0 attention sweep; O-proj tiles interleave into half-1,
which runs chunk-outer with per-chunk normalize so only 4 j-tiles of
O-proj remain after the last exp.
"""

import numpy as np

import concourse.bass as bass
import concourse.tile as tile
from concourse import bacc, mybir
from concourse.bass_utils import run_bass_kernel_spmd

F32 = mybir.dt.float32

B, T, E, H = 4, 2048, 1024, 16
DK = E // H            # 64
N_CORES = 8
FL = 512               # local f (8 heads * 64)
HL = 8                 # heads per core
NT = T // 128          # 16 t-tiles
NE = E // 128          # 8 e-tiles
NFT = FL // 128        # 4 local f-tiles (head pairs)
VW = 72                # per-head width in vt tiles (64 data + 1 ones + pad)

BF16 = mybir.dt.bfloat16
DT = BF16
Exp = mybir.ActivationFunctionType.Exp
MUL = mybir.AluOpType.mult


def build_nc():
    nc = bacc.Bacc("TRN2", target_bir_lowering=False, debug=False,
                   enable_asserts=False)

    qT = nc.dram_tensor("qT", [E, T], DT, kind="ExternalInput").ap()
    kT = nc.dram_tensor("kT", [E, T], DT, kind="ExternalInput").ap()
    vT = nc.dram_tensor("vT", [E, T], DT, kind="ExternalInput").ap()
    wqT = nc.dram_tensor("wqT", [E, FL], DT, kind="ExternalInput").ap()
    wkT = nc.dram_tensor("wkT", [E, FL], DT, kind="ExternalInput").ap()
    wvT = nc.dram_tensor("wvT", [E, FL], DT, kind="ExternalInput").ap()
    woT = nc.dram_tensor("woT", [FL, E], DT, kind="ExternalInput").ap()
    bq = nc.dram_tensor("bq", [128, NFT], F32, kind="ExternalInput").ap()
    bk = nc.dram_tensor("bk", [128, NFT], F32, kind="ExternalInput").ap()
    bv = nc.dram_tensor("bv", [1, FL], DT, kind="ExternalInput").ap()
    bo = nc.dram_tensor("bo", [1, E], DT, kind="ExternalInput").ap()
    ones_d = nc.dram_tensor("ones_d", [1, 128], DT, kind="ExternalInput").ap()
    onesrow = nc.dram_tensor("onesrow", [1, T], DT,
                             kind="ExternalInput").ap()
    maskrow = nc.dram_tensor("maskrow", [1, T], DT,
                             kind="ExternalInput").ap()
    out = nc.dram_tensor("out", [T, E], DT, kind="ExternalOutput").ap()

    with tile.TileContext(nc) as tc:
        with (
            tc.tile_pool(name="const", bufs=1) as constp,
            tc.tile_pool(name="qkt", bufs=1) as qktp,
            tc.tile_pool(name="vsb", bufs=1) as vsbp,
            tc.tile_pool(name="xtl", bufs=1) as xtlp,
            tc.tile_pool(name="wqk", bufs=1) as wp,
            tc.tile_pool(name="wo", bufs=1) as wop,
            tc.tile_pool(name="xload", bufs=20) as xlp,
            tc.tile_pool(name="vload", bufs=8) as vlp,
            tc.tile_pool(name="exps", bufs=8) as expp,
            tc.tile_pool(name="norm", bufs=2) as normp,
            tc.tile_pool(name="osb", bufs=4) as osbp,
            tc.tile_pool(name="ps_s", bufs=2, space="PSUM") as ps_s,
            tc.tile_pool(name="ps_o", bufs=2, space="PSUM") as ps_o,
            tc.tile_pool(name="ps_w", bufs=2, space="PSUM") as ps_w,
        ):
            # ---- constants ----
            bq_sb = constp.tile([128, NFT], F32, tag="bq")
            nc.sync.dma_start(out=bq_sb[:], in_=bq)
            bk_sb = constp.tile([128, NFT], F32, tag="bk")
            nc.sync.dma_start(out=bk_sb[:], in_=bk)
            bv_sb = constp.tile([1, FL], DT, tag="bv")
            nc.sync.dma_start(out=bv_sb[:], in_=bv)
            bo_sb = constp.tile([1, E], DT, tag="bo")
            nc.sync.dma_start(out=bo_sb[:], in_=bo)
            ones_sb = constp.tile([1, 128], DT, tag="ones")
            nc.sync.dma_start(out=ones_sb[:], in_=ones_d)

            # persistent per-head Q^T/K^T tiles: rows 0..63 = head data,
            # row 64 = ones (Q) / scaled mask bias (K)
            qt = [qktp.tile([65, T], DT, tag=f"qt{i}", name=f"qt{i}")
                  for i in range(HL)]
            kt = [qktp.tile([65, T], DT, tag=f"kt{i}", name=f"kt{i}")
                  for i in range(HL)]
            for i in range(HL):
                nc.sync.dma_start(out=qt[i][64:65, :], in_=onesrow)
                nc.sync.dma_start(out=kt[i][64:65, :], in_=maskrow)
            # V per t-tile: [128, 8 heads * 72]; per head: cols 0..63 = V,
            # col 64 = 1.0 (row-sum trick), cols 65..71 unused
            vt = [vsbp.tile([128, HL * VW], DT, tag=f"v{j}", name=f"v{j}")
                  for j in range(NT)]
            xtl = [xtlp.tile([128, T], DT, tag=f"x{i}", name=f"x{i}")
                   for i in range(NFT)]

            # ---- weights ----
            w_sb = {}
            for name, wdram in (("k", wkT), ("q", wqT)):
                w_sb[name] = [
                    wp.tile([128, FL], DT, tag=f"w{name}{e}",
                            name=f"w{name}{e}") for e in range(NE)]
                for e in range(NE):
                    nc.sync.dma_start(
                        out=w_sb[name][e][:],
                        in_=wdram[e * 128:(e + 1) * 128, :])
            wv_sb = [wp.tile([128, FL], DT, tag=f"wv{e}", name=f"wv{e}")
                     for e in range(NE)]
            for e in range(NE):
                nc.sync.dma_start(out=wv_sb[e][:],
                                  in_=wvT[e * 128:(e + 1) * 128, :])
            wo_sb = [wop.tile([128, E], DT, tag=f"wo{e}", name=f"wo{e}")
                     for e in range(NFT)]
            for e in range(NFT):
                nc.sync.dma_start(out=wo_sb[e][:],
                                  in_=woT[e * 128:(e + 1) * 128, :])

            # ---------- emission helpers ----------

            def v_proj_units():
                """V projection, one yield-unit per t-tile. vT streamed
                in [128,512] quarters, DMA issued from the Pool queue."""
                for qtr in range(4):
                    vf = [vlp.tile([128, 512], DT, tag="vf", name="vf")
                          for _ in range(NE)]
                    for e in range(NE):
                        nc.sync.dma_start(
                            out=vf[e][:],
                            in_=vT[e * 128:(e + 1) * 128,
                                   qtr * 512:(qtr + 1) * 512])
                    for jj in range(4):
                        j = qtr * 4 + jj
                        ps = ps_w.tile([128, 512], F32, tag="psw",
                                       name="psv")
                        for e in range(NE):
                            nc.tensor.matmul(
                                ps[:],
                                lhsT=vf[e][:, jj * 128:(jj + 1) * 128],
                                rhs=wv_sb[e][:],
                                start=(e == 0), stop=False)
                        nc.tensor.matmul(ps[:], lhsT=ones_sb[:],
                                         rhs=bv_sb[:], start=False, stop=True)
                        nc.vector.tensor_copy(
                            out=vt[j].rearrange(
                                "p (h w) -> p h w", w=VW)[:, :, 0:64],
                            in_=ps[:].rearrange(
                                "p (h w) -> p h w", w=64))
                        nc.gpsimd.memset(
                            vt[j].rearrange(
                                "p (h w) -> p h w", w=VW)[:, :, 64:65],
                            1.0)
                        yield

            def kq_proj_units(fs):
                """K/Q projection, pair-major (pair p completes before
                pair p+1 starts; x chunks re-streamed per pair). DMAs for
                group (f,c+1) are issued before group (f,c)'s units so
                loads stay a group ahead. One yield-unit = one (f, c,
                name) psum tile."""
                groups = [(f, c) for f in fs for c in range(4)]

                def load(f, c):
                    xs = {}
                    for nm, xdram in (("k", kT), ("q", qT)):
                        xs[nm] = []
                        for e in range(NE):
                            xe = xlp.tile([128, 512], DT, tag="xchunk",
                                          name="xchunk")
                            nc.sync.dma_start(
                                out=xe[:],
                                in_=xdram[e * 128:(e + 1) * 128,
                                          c * 512:(c + 1) * 512])
                            xs[nm].append(xe)
                    return xs

                pending = load(*groups[0])
                for gi, (f, c) in enumerate(groups):
                    xs = pending
                    if gi + 1 < len(groups):
                        pending = load(*groups[gi + 1])
                    for nm in ("k", "q"):
                        bias_sb = bk_sb if nm == "k" else bq_sb
                        dsts = kt if nm == "k" else qt
                        ps = ps_w.tile([128, 512], F32, tag="psw",
                                       name="psw")
                        for e in range(NE):
                            nc.tensor.matmul(
                                ps[:],
                                lhsT=w_sb[nm][e][:, f * 128:(f + 1) * 128],
                                rhs=xs[nm][e][:],
                                start=(e == 0), stop=(e == NE - 1))
                        for hh in range(2):
                            nc.vector.tensor_scalar_add(
                                dsts[2 * f + hh][0:64,
                                                 c * 512:(c + 1) * 512],
                                ps[hh * 64:(hh + 1) * 64, :],
                                bias_sb[hh * 64:(hh + 1) * 64, f:f + 1])
                        yield

            def o_proj_units(js):
                """Output projection. One yield-unit = one (j, c2); the
                bf16 partial store DMA is issued per 512-col chunk."""
                for j in js:
                    for c2 in range(2):
                        ps = ps_w.tile([128, 512], F32, tag="psw",
                                       name="psf")
                        nc.tensor.matmul(
                            ps[:], lhsT=ones_sb[:],
                            rhs=bo_sb[:, c2 * 512:(c2 + 1) * 512],
                            start=True, stop=False)
                        for e in range(NFT):
                            nc.tensor.matmul(
                                ps[:],
                                lhsT=xtl[e][:, j * 128:(j + 1) * 128],
                                rhs=wo_sb[e][:, c2 * 512:(c2 + 1) * 512],
                                start=False, stop=(e == NFT - 1))
                        ob = osbp.tile([128, 512], DT, tag="ob", name="ob")
                        nc.vector.tensor_copy(out=ob[:], in_=ps[:])
                        nc.sync.dma_start(
                            out=out[j * 128:(j + 1) * 128,
                                    c2 * 512:(c2 + 1) * 512],
                            in_=ob[:])
                        yield

            # ---- attention: software-pipelined scores/exp/av stream ----
            # one unit = (head, global q-chunk gc, k-tile-pair kp):
            # 2 score matmuls [128,512] into one [128,1024] psum tile,
            # 1 exp, 2 av matmuls accumulating into pso [65,512].

            def att_chunk_units(h, gc):
                qh = qt[h]
                kh = kt[h]
                q0 = gc * 512
                pso = ps_o.tile([65, 512], F32, tag="pso", name="pso")
                for kp in range(NT // 2):
                    st = {}

                    def scores(kp=kp, st=st):
                        ps = ps_s.tile([128, 1024], F32, tag="ps_s",
                                       name="pss")
                        for i in range(2):
                            k = 2 * kp + i
                            nc.tensor.matmul(
                                ps[:, i * 512:(i + 1) * 512],
                                lhsT=kh[:, k * 128:(k + 1) * 128],
                                rhs=qh[:, q0:q0 + 512],
                                start=True, stop=True)
                        st["ps"] = ps

                    def exp_av(kp=kp, st=st, pso=pso, h=h):
                        es = expp.tile([128, 1024], DT, tag="es",
                                       name="es")
                        nc.scalar.activation(
                            out=es[:], in_=st["ps"][:], func=Exp,
                            scale=0.125)
                        for i in range(2):
                            k = 2 * kp + i
                            nc.tensor.matmul(
                                pso[:],
                                lhsT=vt[k][:, h * VW:h * VW + 65],
                                rhs=es[:, i * 512:(i + 1) * 512],
                                start=(kp == 0 and i == 0),
                                stop=(kp == NT // 2 - 1 and i == 1))
                    yield scores, exp_av, pso

            def normalize_cols(h, q0, w, oT, zraw):
                """xtl[...] <- oT[:, :w] / zraw[:, :w], written to q-cols
                q0:q0+w. zraw is partition-0 based (custom-DVE recip
                breaks on partition-offset inputs)."""
                rep = normp.tile([64, 1024], F32, tag="rep", name="rep")
                nc.vector.reciprocal_approx_fast(rep[0:1, 0:w],
                                                 zraw[:, 0:w])
                nc.gpsimd.partition_broadcast(rep[:, 0:w], rep[0:1, 0:w])
                f = h // 2
                r0 = (h % 2) * 64
                nc.vector.tensor_tensor(
                    out=xtl[f][r0:r0 + 64, q0:q0 + w],
                    in0=oT[:, 0:w], in1=rep[:, 0:w], op=MUL)

            # pipeline driver: emits scores(u), then exp+av(u-1), then
            # filler PE units at their rates.
            class Pipe:
                def __init__(self):
                    self.prev = None

                def run_unit(self, scores, tail, fillers, credit):
                    scores()
                    if self.prev is not None:
                        self.prev()
                    for fi, (fill, rate) in enumerate(fillers):
                        credit[fi] += rate
                        while credit[fi] >= 1.0:
                            credit[fi] -= 1.0
                            try:
                                next(fill)
                            except StopIteration:
                                credit[fi] = 0.0
                                break
                    self.prev = tail

                def flush(self):
                    if self.prev is not None:
                        self.prev()
                    self.prev = None

            def sweep_chunk(pipe, h, gc, oT, zraw, cc, fillers, credit,
                            norm=None):
                """Emit one (head, chunk) group of 8 units; stage pso
                into oT[:, cc*512:...] / zraw[:, cc*512:...] after the
                last unit's exp+av. `norm`, if given, is emitted right
                after the stage (lazily, inside the pipelined tail, so
                the PE stream is not broken at head boundaries)."""
                units = list(att_chunk_units(h, gc))
                for kp, (scores, exp_av, pso) in enumerate(units):
                    if kp == NT // 2 - 1:
                        def tail(ea=exp_av, pso=pso, cc=cc, oT=oT,
                                 zraw=zraw, norm=norm):
                            ea()
                            nc.vector.tensor_copy(
                                out=oT[:, cc * 512:(cc + 1) * 512],
                                in_=pso[0:64, :])
                            nc.vector.tensor_copy(
                                out=zraw[:, cc * 512:(cc + 1) * 512],
                                in_=pso[64:65, :])
                            if norm is not None:
                                norm()
                    else:
                        tail = exp_av
                    pipe.run_unit(scores, tail, fillers, credit)

            # ---------- schedule ----------
            # KQ pair 0 leads (it gates the first scores; its DMAs are
            # issued first). V-proj interleaves as a high-rate filler at
            # the front of the half-0 sweep; KQ pairs 1-3 follow (pair p
            # completes well before head 2p starts).
            for _ in kq_proj_units([0]):
                pass

            pipe = Pipe()
            f_v = v_proj_units()
            f_kq = kq_proj_units([1, 2, 3])
            fillers = [(f_v, 2.0), (f_kq, 0.4)]
            credit = [0.0, 0.0]
            for h in range(HL):
                oT = normp.tile([64, 1024], F32, tag="oT", name="oT")
                zraw = normp.tile([1, 1024], F32, tag="zraw", name="zraw")
                sweep_chunk(pipe, h, 0, oT, zraw, 0, fillers, credit)
                sweep_chunk(pipe, h, 1, oT, zraw, 1, fillers, credit,
                            norm=lambda h=h, oT=oT, zraw=zraw:
                            normalize_cols(h, 0, 1024, oT, zraw))
            for f in (f_v, f_kq):
                for _ in f:
                    pass

            # q-half 1, chunk-outer with per-chunk normalize; O-proj
            # j0-7 (half-0 cols) fills chunk-2's sweep, j8-11 fills
            # chunk-3's, j12-15 is the only tail.
            f_o1 = o_proj_units(range(8))
            fillers = [(f_o1, 0.25)]
            credit = [0.0]
            for h in range(HL):
                oT = normp.tile([64, 1024], F32, tag="oT", name="oT")
                zraw = normp.tile([1, 1024], F32, tag="zraw", name="zraw")
                sweep_chunk(pipe, h, 2, oT, zraw, 0, fillers, credit,
                            norm=lambda h=h, oT=oT, zraw=zraw:
                            normalize_cols(h, 1024, 512, oT, zraw))
            for _ in f_o1:
                pass

            f_o2 = o_proj_units(range(8, 12))
            fillers = [(f_o2, 0.125)]
            credit = [0.0]
            for h in range(HL):
                oT = normp.tile([64, 1024], F32, tag="oT", name="oT")
                zraw = normp.tile([1, 1024], F32, tag="zraw", name="zraw")
                sweep_chunk(pipe, h, 3, oT, zraw, 0, fillers, credit,
                            norm=lambda h=h, oT=oT, zraw=zraw:
                            normalize_cols(h, 1536, 512, oT, zraw))
            pipe.flush()
            for _ in f_o2:
                pass
            for _ in o_proj_units(range(12, NT)):
                pass

    nc.compile()
    return nc


_NC_CACHE = None


def _get_nc():
    global _NC_CACHE
    if _NC_CACHE is None:
        _NC_CACHE = build_nc()
    return _NC_CACHE


def make_in_maps(query, key_, value, mask, w_q, b_q, w_k, b_k, w_v, b_v,
                 w_o, b_o):
    import ml_dtypes
    f32 = np.float32
    bf16 = ml_dtypes.bfloat16
    c = lambda a: np.ascontiguousarray(a).astype(bf16)
    in_maps = []
    for core in range(N_CORES):
        b, g = core // 2, core % 2
        fs = slice(g * FL, (g + 1) * FL)
        # row-64 mask bias, pre-multiplied by 8 (exp applies scale 1/8)
        mrow = np.where(mask[b], 0.0, -240.0).astype(f32)
        in_maps.append({
            "qT": c(query[b].T.astype(f32, copy=False)),
            "kT": c(key_[b].T.astype(f32, copy=False)),
            "vT": c(value[b].T.astype(f32, copy=False)),
            "wqT": c(w_q[fs, :].T.astype(f32, copy=False)),
            "wkT": c(w_k[fs, :].T.astype(f32, copy=False)),
            "wvT": c(w_v[fs, :].T.astype(f32, copy=False)),
            "woT": c(w_o[:, fs].T.astype(f32, copy=False)),
            "bq": np.ascontiguousarray(
                b_q[fs].astype(f32, copy=False).reshape(NFT, 128).T),
            "bk": np.ascontiguousarray(
                b_k[fs].astype(f32, copy=False).reshape(NFT, 128).T),
            "bv": b_v[fs].reshape(1, FL).astype(bf16),
            "bo": (b_o.astype(f32, copy=False) if g == 0
                   else np.zeros(E, f32)).reshape(1, E).astype(bf16),
            "ones_d": np.ones((1, 128), bf16),
            "onesrow": np.ones((1, T), bf16),
            "maskrow": mrow.reshape(1, T).astype(bf16),
        })
    return in_maps


def kernel(query=None, key_=None, value=None, mask=None, w_q=None, b_q=None,
           w_k=None, b_k=None, w_v=None, b_v=None, w_o=None, b_o=None,
           key=None, **_kwargs):
    if key_ is None:
        key_ = key
    args = [np.asarray(a) for a in
            (query, key_, value, mask, w_q, b_q, w_k, b_k, w_v, b_v,
             w_o, b_o)]
    nc = _get_nc()
    in_maps = make_in_maps(*args)
    res = run_bass_kernel_spmd(nc, in_maps, core_ids=list(range(N_CORES)))
    outs = [np.asarray(res.results[i]["out"], dtype=np.float32)
            for i in range(N_CORES)]
    full = np.empty((B, T, E), np.float32)
    for b in range(B):
        full[b] = outs[2 * b] + outs[2 * b + 1]
    return full
